# revision 10
# baseline (speedup 1.0000x reference)
"""Trainium2 Bass kernel for nn_Blur: 4x4 FIR depthwise blur with pad (2,1).

out[n,c,i,j] = sum_{a,b} K[a,b] * x[n,c, i+1-a, j+1-b]   (zero-padded)

Strategy (8 NeuronCores, pure data parallelism over the 8192 (n,c) slices):
  - Each core processes 1024 slices of 64x64.
  - SBUF layout per tile: partition p = m*64 + h (m in {0,1} packs two groups
    of 32 slices so the full 128-wide PE contraction is used), free = (s, w).
  - The H-convolution lives in 4 banded [128,128] stationary matrices (one
    per W-tap b): lhsT_b[u + 64m, i + 64m'] = delta(m,m') * K[i-u+1, b].
  - The W-convolution comes from 4 PSUM-accumulated matmuls whose rhs is the
    same tile shifted along the free (W) dim; a 3-column zero pad makes all
    four matmuls full-range N=512.
  - float32r matmuls run at full PE rate for N>=256.
"""

import numpy as np

import concourse.bacc as bacc
import concourse.mybir as mybir
from concourse.tile import TileContext
from concourse.bass_utils import run_bass_kernel_spmd

N_CORES = 8
B, C, H, W = 32, 256, 64, 64
NSLICES = B * C                      # 8192
SLICES_PER_CORE = NSLICES // N_CORES  # 1024
TILE_SLICES = 64                     # slices per SBUF tile (2 members x 32)
SG = TILE_SLICES // 2                # s-groups per member = 32
NTILES = SLICES_PER_CORE // TILE_SLICES  # 16
WPAD = W + 3                         # 2 left zero cols + 1 right zero col
F32 = mybir.dt.float32
F32R = mybir.dt.float32r

_NC_CACHE = {}


def _build_wmat(K: np.ndarray) -> np.ndarray:
    """(4, 128, 128) fp32: per-W-tap block-diag transposed H-band matrices."""
    K = np.asarray(K, np.float32)
    wmat = np.zeros((4, 128, 128), np.float32)
    for b in range(4):
        T = np.zeros((H, H), np.float32)
        for i in range(H):
            for u in range(max(0, i - 2), min(H, i + 2)):
                T[i, u] = K[i - u + 1, b]
        lhsT = T.T  # lhsT[u, i] = K[i-u+1, b]
        wmat[b, :H, :H] = lhsT
        wmat[b, H:, H:] = lhsT
    return wmat


def _build_nc(slices_per_core: int = SLICES_PER_CORE):
    ntiles = slices_per_core // TILE_SLICES
    nc = bacc.Bacc("TRN2", target_bir_lowering=False, debug=False)
    # Inputs are declared float32r (same bits as fp32 host-side): every DMA
    # is then same-dtype, and the BIR verifier sees fp32r producers for the
    # fp32r matmuls. x arrives host-padded to WPAD (2 zero cols left, 1
    # right) so no on-chip memset of the fp32r tile is needed.
    x = nc.dram_tensor("x", [slices_per_core, H, WPAD], F32R, kind="ExternalInput").ap()
    wm = nc.dram_tensor("w", [4, 128, 128], F32R, kind="ExternalInput").ap()
    y = nc.dram_tensor("y", [slices_per_core, H, W], F32, kind="ExternalOutput").ap()

    with TileContext(nc) as tc:
        with (
            tc.tile_pool(name="wpool", bufs=1) as wpool,
            tc.tile_pool(name="xpool", bufs=3) as xpool,
            tc.tile_pool(name="opool", bufs=3) as opool,
            tc.tile_pool(name="pspool", bufs=8, space="PSUM") as pspool,
        ):
            wsb = wpool.tile([128, 4, 128], F32R, name="wsb")
            nc.sync.dma_start(wsb[:], wm.rearrange("b k m -> k b m"))

            for t in range(ntiles):
                xt = xpool.tile([128, SG, WPAD], F32R, name="xt")
                xin = x[t * TILE_SLICES : (t + 1) * TILE_SLICES]
                # partition p = m*64 + h with member m = slice parity, so the
                # (m h) partition dim is a single stride-WPAD run in DRAM
                nc.sync.dma_start(
                    xt[:],
                    xin.rearrange("(s m) h w -> (m h) s w", m=2),
                )

                ot = opool.tile([128, SG, W], F32, name="ot")
                for q in range(SG // 8):
                    ps = pspool.tile([128, 512], F32, name="ps")
                    for b in range(4):
                        c0 = 3 - b
                        rhs = xt[:, 8 * q : 8 * q + 8, c0 : c0 + W]
                        nc.tensor.matmul(
                            ps[:],
                            wsb[:, b, :],
                            rhs,
                            start=(b == 0),
                            stop=(b == 3),
                        )
                    nc.scalar.copy(ot[:, 8 * q : 8 * q + 8, :], ps[:])

                yout = y[t * TILE_SLICES : (t + 1) * TILE_SLICES]
                nc.sync.dma_start(
                    yout.rearrange("(s m) h w -> (m h) s w", m=2),
                    ot[:],
                )

    nc.compile()
    return nc


def get_nc(slices_per_core: int = SLICES_PER_CORE):
    if slices_per_core not in _NC_CACHE:
        _NC_CACHE[slices_per_core] = _build_nc(slices_per_core)
    return _NC_CACHE[slices_per_core]


def kernel(x: np.ndarray, kernel: np.ndarray, _trace: bool = False, **_tkw):
    x = np.asarray(x, np.float32)
    wmat = _build_wmat(kernel)
    b, c, h, w = x.shape
    xs = np.zeros((b * c, h, WPAD), np.float32)
    xs[:, :, 2 : 2 + w] = x.reshape(b * c, h, w)
    spc = (b * c) // N_CORES
    nc = get_nc(spc)
    in_maps = [
        {"x": xs[k * spc : (k + 1) * spc], "w": wmat} for k in range(N_CORES)
    ]
    res = run_bass_kernel_spmd(
        nc, in_maps, list(range(N_CORES)), trace=_trace, **_tkw
    )
    out = np.concatenate([res.results[k]["y"] for k in range(N_CORES)], axis=0)
    result = out.reshape(b, c, h, w)
    if _trace:
        return result, res
    return result


# revision 12
# speedup vs baseline: 1.3091x; 1.3091x over previous
"""Trainium2 Bass kernel for nn_Blur: 4x4 FIR depthwise blur with pad (2,1).

out[n,c,i,j] = sum_{a,b} K[a,b] * x[n,c, i+1-a, j+1-b]   (zero-padded)

Strategy (8 NeuronCores, pure data parallelism over the 8192 (n,c) slices):
  - Each core processes 1024 slices of 64x64.
  - SBUF layout per tile of 64 slices: partition p = m*64 + h (member
    m = slice parity packs two slices so the full 128-wide PE contraction
    is used), free = (s, w) with w zero-padded to 67 (2 left + 1 right).
  - The H-convolution lives in 4 banded [128,128] stationary matrices (one
    per W-tap b): lhsT_b[u + 64m, i + 64m'] = delta(m,m') * K[i-u+1, b].
  - The W-convolution comes from 4 PSUM-accumulated matmuls whose rhs is
    the same tile shifted along the free (W) dim; the zero pad makes all
    four matmuls full-range N=512.
  - float32r matmuls run at full PE rate for N>=256; inputs are declared
    float32r (same bits host-side) so every DMA is same-dtype.
  - The host pre-permutes each core's shard into the exact SBUF tile
    layout, so every DMA descriptor is one contiguous run per partition
    (line-rate HBM instead of 256B-descriptor-limited).
"""

import numpy as np

import concourse.bacc as bacc
import concourse.mybir as mybir
from concourse.tile import TileContext
from concourse.bass_utils import run_bass_kernel_spmd

N_CORES = 8
B, C, H, W = 32, 256, 64, 64
NSLICES = B * C                      # 8192
SLICES_PER_CORE = NSLICES // N_CORES  # 1024
TILE_SLICES = 64                     # slices per SBUF tile (2 members x 32)
SG = TILE_SLICES // 2                # s-groups per member = 32
WPAD = W + 3                         # 2 left zero cols + 1 right zero col
F32 = mybir.dt.float32
F32R = mybir.dt.float32r

_NC_CACHE = {}


def _build_wmat(K: np.ndarray) -> np.ndarray:
    """(4, 128, 128) fp32: per-W-tap block-diag transposed H-band matrices."""
    K = np.asarray(K, np.float32)
    wmat = np.zeros((4, 128, 128), np.float32)
    for b in range(4):
        T = np.zeros((H, H), np.float32)
        for i in range(H):
            for u in range(max(0, i - 2), min(H, i + 2)):
                T[i, u] = K[i - u + 1, b]
        lhsT = T.T  # lhsT[u, i] = K[i-u+1, b]
        wmat[b, :H, :H] = lhsT
        wmat[b, H:, H:] = lhsT
    return wmat


def _build_nc(slices_per_core: int = SLICES_PER_CORE):
    ntiles = slices_per_core // TILE_SLICES
    nc = bacc.Bacc("TRN2", target_bir_lowering=False, debug=False)
    # DRAM layouts are the SBUF tile layouts (host pre-/post-permutes):
    #   x: [tile, p=(m h), (s w')]  with w' zero-padded to WPAD
    #   y: [tile, p=(m h), (s w)]
    x = nc.dram_tensor(
        "x", [ntiles, 128, SG * WPAD], F32R, kind="ExternalInput"
    ).ap()
    wm = nc.dram_tensor("w", [4, 128, 128], F32R, kind="ExternalInput").ap()
    y = nc.dram_tensor(
        "y", [ntiles, 128, SG * W], F32, kind="ExternalOutput"
    ).ap()

    with TileContext(nc) as tc:
        with (
            tc.tile_pool(name="wpool", bufs=1) as wpool,
            tc.tile_pool(name="xpool", bufs=4) as xpool,
            tc.tile_pool(name="opool", bufs=4) as opool,
            tc.tile_pool(name="pspool", bufs=8, space="PSUM") as pspool,
        ):
            wsb = wpool.tile([128, 4, 128], F32R, name="wsb")
            nc.sync.dma_start(wsb[:], wm.rearrange("b k m -> k b m"))

            for t in range(ntiles):
                xt = xpool.tile([128, SG, WPAD], F32R, name="xt")
                nc.sync.dma_start(xt[:], x[t])

                ot = opool.tile([128, SG, W], F32, name="ot")
                for q in range(SG // 8):
                    ps = pspool.tile([128, 512], F32, name="ps")
                    for b in range(4):
                        c0 = 3 - b
                        rhs = xt[:, 8 * q : 8 * q + 8, c0 : c0 + W]
                        nc.tensor.matmul(
                            ps[:],
                            wsb[:, b, :],
                            rhs,
                            start=(b == 0),
                            stop=(b == 3),
                        )
                    # alternate copy engine: DVE and ACT are both idle-ish
                    if q % 2 == 0:
                        nc.vector.tensor_copy(ot[:, 8 * q : 8 * q + 8, :], ps[:])
                    else:
                        nc.scalar.copy(ot[:, 8 * q : 8 * q + 8, :], ps[:])

                nc.sync.dma_start(y[t], ot[:])

    nc.compile()
    return nc


def get_nc(slices_per_core: int = SLICES_PER_CORE):
    if slices_per_core not in _NC_CACHE:
        _NC_CACHE[slices_per_core] = _build_nc(slices_per_core)
    return _NC_CACHE[slices_per_core]


def _pack_input(xs: np.ndarray) -> np.ndarray:
    """[S, H, W] fp32 -> [S/64, 128, SG*WPAD] in the SBUF tile layout."""
    s = xs.shape[0]
    ntiles = s // TILE_SLICES
    xp = np.zeros((s, H, WPAD), np.float32)
    xp[:, :, 2 : 2 + W] = xs
    # (t, s, m, h, w) -> (t, m, h, s, w)
    v = xp.reshape(ntiles, SG, 2, H, WPAD).transpose(0, 2, 3, 1, 4)
    return np.ascontiguousarray(v.reshape(ntiles, 128, SG * WPAD))


def _unpack_output(yp: np.ndarray) -> np.ndarray:
    """[S/64, 128, SG*W] -> [S, H, W]."""
    ntiles = yp.shape[0]
    v = yp.reshape(ntiles, 2, H, SG, W).transpose(0, 3, 1, 2, 4)
    return v.reshape(ntiles * TILE_SLICES, H, W)


def kernel(x: np.ndarray, kernel: np.ndarray, _trace: bool = False, **_tkw):
    x = np.asarray(x, np.float32)
    wmat = _build_wmat(kernel)
    b, c, h, w = x.shape
    xs = x.reshape(b * c, h, w)
    spc = (b * c) // N_CORES
    nc = get_nc(spc)
    in_maps = [
        {"x": _pack_input(xs[k * spc : (k + 1) * spc]), "w": wmat}
        for k in range(N_CORES)
    ]
    res = run_bass_kernel_spmd(
        nc, in_maps, list(range(N_CORES)), trace=_trace, **_tkw
    )
    out = np.concatenate(
        [_unpack_output(res.results[k]["y"]) for k in range(N_CORES)], axis=0
    )
    result = out.reshape(b, c, h, w)
    if _trace:
        return result, res
    return result


# revision 13
# speedup vs baseline: 1.4776x; 1.1288x over previous
"""Trainium2 Bass kernel for nn_Blur: 4x4 FIR depthwise blur with pad (2,1).

out[n,c,i,j] = sum_{a,b} K[a,b] * x[n,c, i+1-a, j+1-b]   (zero-padded)

Strategy (8 NeuronCores, pure data parallelism over the 8192 (n,c) slices):
  - Each core processes 1024 slices of 64x64.
  - SBUF layout per tile of 64 slices: partition p = m*64 + h (member
    m = slice parity packs two slices so the full 128-wide PE contraction
    is used), free = (s, w) with w zero-padded to 67 (2 left + 1 right).
  - The H-convolution lives in 4 banded [128,128] stationary matrices (one
    per W-tap b): lhsT_b[u + 64m, i + 64m'] = delta(m,m') * K[i-u+1, b].
  - The W-convolution comes from 4 PSUM-accumulated matmuls whose rhs is
    the same tile shifted along the free (W) dim; the zero pad makes all
    four matmuls full-range N=512.
  - float32r matmuls run at full PE rate for N>=256; inputs are declared
    float32r (same bits host-side) so every DMA is same-dtype.
  - The host pre-permutes each core's shard into the exact SBUF tile
    layout, so every DMA descriptor is one contiguous run per partition
    (line-rate HBM instead of 256B-descriptor-limited).
"""

import numpy as np

import concourse.bacc as bacc
import concourse.mybir as mybir
from concourse.tile import TileContext
from concourse.bass_utils import run_bass_kernel_spmd

N_CORES = 8
B, C, H, W = 32, 256, 64, 64
NSLICES = B * C                      # 8192
SLICES_PER_CORE = NSLICES // N_CORES  # 1024
TILE_SLICES = 64                     # slices per SBUF tile (2 members x 32)
SG = TILE_SLICES // 2                # s-groups per member = 32
WPAD = W + 3                         # 2 left zero cols + 1 right zero col
F32 = mybir.dt.float32
F32R = mybir.dt.float32r

_NC_CACHE = {}


def _build_wmat(K: np.ndarray) -> np.ndarray:
    """(4, 128, 128) fp32: per-W-tap block-diag transposed H-band matrices."""
    K = np.asarray(K, np.float32)
    wmat = np.zeros((4, 128, 128), np.float32)
    for b in range(4):
        T = np.zeros((H, H), np.float32)
        for i in range(H):
            for u in range(max(0, i - 2), min(H, i + 2)):
                T[i, u] = K[i - u + 1, b]
        lhsT = T.T  # lhsT[u, i] = K[i-u+1, b]
        wmat[b, :H, :H] = lhsT
        wmat[b, H:, H:] = lhsT
    return wmat


def _build_nc(slices_per_core: int = SLICES_PER_CORE):
    ntiles = slices_per_core // TILE_SLICES
    nc = bacc.Bacc("TRN2", target_bir_lowering=False, debug=False)
    # DRAM layouts are the SBUF tile layouts (host pre-/post-permutes):
    #   x: [tile, p=(m h), (s w')]  with w' zero-padded to WPAD
    #   y: [tile, p=(m h), (s w)]
    x = nc.dram_tensor(
        "x", [ntiles, 128, SG * WPAD], F32R, kind="ExternalInput"
    ).ap()
    wm = nc.dram_tensor("w", [4, 128, 128], F32R, kind="ExternalInput").ap()
    y = nc.dram_tensor(
        "y", [ntiles, 128, SG * W], F32, kind="ExternalOutput"
    ).ap()

    with TileContext(nc) as tc:
        with (
            tc.tile_pool(name="wpool", bufs=1) as wpool,
            tc.tile_pool(name="xpool", bufs=4) as xpool,
            tc.tile_pool(name="opool", bufs=4) as opool,
            tc.tile_pool(name="pspool", bufs=8, space="PSUM") as pspool,
        ):
            wsb = wpool.tile([128, 4, 128], F32R, name="wsb")
            nc.sync.dma_start(wsb[:], wm.rearrange("b k m -> k b m"))

            for t in range(ntiles):
                xt = xpool.tile([128, SG, WPAD], F32R, name="xt")
                nc.sync.dma_start(xt[:], x[t])

                ot = opool.tile([128, SG, W], F32, name="ot")
                for q in range(SG // 8):
                    ps = pspool.tile([128, 512], F32, name="ps")
                    for b in range(4):
                        c0 = 3 - b
                        rhs = xt[:, 8 * q : 8 * q + 8, c0 : c0 + W]
                        nc.tensor.matmul(
                            ps[:],
                            wsb[:, b, :],
                            rhs,
                            start=(b == 0),
                            stop=(b == 3),
                        )
                    # alternate copy engine: DVE and ACT are both idle-ish
                    if q % 2 == 0:
                        nc.vector.tensor_copy(ot[:, 8 * q : 8 * q + 8, :], ps[:])
                    else:
                        nc.scalar.copy(ot[:, 8 * q : 8 * q + 8, :], ps[:])

                # separate HWDGE ring (ACT) so output stores never
                # head-of-line-block the SP ring feeding input loads
                nc.scalar.dma_start(y[t], ot[:])

    nc.compile()
    return nc


def get_nc(slices_per_core: int = SLICES_PER_CORE):
    if slices_per_core not in _NC_CACHE:
        _NC_CACHE[slices_per_core] = _build_nc(slices_per_core)
    return _NC_CACHE[slices_per_core]


def _pack_input(xs: np.ndarray) -> np.ndarray:
    """[S, H, W] fp32 -> [S/64, 128, SG*WPAD] in the SBUF tile layout."""
    s = xs.shape[0]
    ntiles = s // TILE_SLICES
    xp = np.zeros((s, H, WPAD), np.float32)
    xp[:, :, 2 : 2 + W] = xs
    # (t, s, m, h, w) -> (t, m, h, s, w)
    v = xp.reshape(ntiles, SG, 2, H, WPAD).transpose(0, 2, 3, 1, 4)
    return np.ascontiguousarray(v.reshape(ntiles, 128, SG * WPAD))


def _unpack_output(yp: np.ndarray) -> np.ndarray:
    """[S/64, 128, SG*W] -> [S, H, W]."""
    ntiles = yp.shape[0]
    v = yp.reshape(ntiles, 2, H, SG, W).transpose(0, 3, 1, 2, 4)
    return v.reshape(ntiles * TILE_SLICES, H, W)


def kernel(x: np.ndarray, kernel: np.ndarray, _trace: bool = False, **_tkw):
    x = np.asarray(x, np.float32)
    wmat = _build_wmat(kernel)
    b, c, h, w = x.shape
    xs = x.reshape(b * c, h, w)
    spc = (b * c) // N_CORES
    nc = get_nc(spc)
    in_maps = [
        {"x": _pack_input(xs[k * spc : (k + 1) * spc]), "w": wmat}
        for k in range(N_CORES)
    ]
    res = run_bass_kernel_spmd(
        nc, in_maps, list(range(N_CORES)), trace=_trace, **_tkw
    )
    out = np.concatenate(
        [_unpack_output(res.results[k]["y"]) for k in range(N_CORES)], axis=0
    )
    result = out.reshape(b, c, h, w)
    if _trace:
        return result, res
    return result
